# revision 1
# baseline (speedup 1.0000x reference)
"""Trainium2 Bass kernel for the 6-layer transformer LM (B=4, T=1024, E=1024,
H=16, V=32000), 8 NeuronCores.

Strategy: tensor-parallel over 8 cores (2 heads each; FFN hidden and vocab
split 8-way).  Three compiled SPMD programs (attention block, FFN block,
LM head) launched per layer; the host performs the all-reduce of partial
projections plus residual adds between launches.  LayerNorm is folded into
the matmuls algebraically: ln(x)@W == ((x@W) - mean[t]*colsum(W)) * rstd[t],
with the gain pre-folded into W host-side; the mean correction is a rank-1
PSUM matmul and the rstd multiply rides the PSUM->SBUF copy.

Matmul operands are bf16 (fp32 accumulation in PSUM); stats/softmax
normalization in fp32.
"""
import os
import sys
sys.path.insert(0, "/opt/trn_rl_repo")

"""Bass/Tile programs for the 6-layer transformer LM, TP8 multi-launch design.

Layout conventions:
  - Activations feature-major in DRAM: x [B, E, T] (E on partitions when tiled).
  - LayerNorm folded into matmuls: ln(x) @ W == ((x @ W) - m[t]*colsum(W)) * rstd[t],
    with gain g pre-folded into W on the host (W_eff = g[:,None] * W).
  - All matmul operands in DT (bf16 or float32r); PSUM/stats in fp32.
"""
import numpy as np
import ml_dtypes

import concourse.bacc as bacc
import concourse.tile as tile
from concourse import mybir
from concourse.bass import ts, ds

P = 128
B, T, E, H, HD, V, L = 4, 1024, 1024, 16, 64, 32000, 6
EPS = 1e-5
AF = mybir.ActivationFunctionType

F32 = mybir.dt.float32


def np_dt(dt):
    return ml_dtypes.bfloat16 if dt == mybir.dt.bfloat16 else np.float32


def _stats_rows(nc, pools, x_tile, xsq_tile, ones_dt, eps_tile, dt, n_feat=E):
    """Row-form LN stats for a [128, KO, 512] feature-major chunk.

    Returns (m_dt [1,512] DT, rstd_row [1,512] f32, mean_f32 [1,512], rstd... )
    """
    stats_ps, small = pools["stats_psum"], pools["small"]
    KO = x_tile.shape[1]
    TW = x_tile.shape[2]
    ps_sum = stats_ps.tile([1, TW], F32, tag="ps_small", name="ps_sum")
    ps_sq = stats_ps.tile([1, TW], F32, tag="ps_small", name="ps_sq")
    for ko in range(KO):
        nc.tensor.matmul(ps_sum[:], ones_dt[:], x_tile[:, ko], start=(ko == 0), stop=(ko == KO - 1))
    for ko in range(KO):
        nc.tensor.matmul(ps_sq[:], ones_dt[:], xsq_tile[:, ko], start=(ko == 0), stop=(ko == KO - 1))
    inv = 1.0 / n_feat
    mean = small.tile([1, TW], F32, tag="mean")
    nc.vector.tensor_scalar_mul(mean[:], ps_sum[:], inv)
    var = small.tile([1, TW], F32, tag="var")
    # var = E[x^2] - mean^2  (no cancellation risk: x is ~unit-scale)
    nc.vector.tensor_scalar_mul(var[:], ps_sq[:], inv)
    msq = small.tile([1, TW], F32, tag="msq")
    nc.vector.tensor_mul(msq[:], mean[:], mean[:])
    nc.vector.tensor_sub(var[:], var[:], msq[:])
    std = small.tile([1, TW], F32, tag="std")
    nc.scalar.activation(std[:], var[:], AF.Sqrt, bias=eps_tile[:1])
    rstd = small.tile([1, TW], F32, tag="rstd")
    nc.vector.reciprocal_approx_fast(out=rstd[:], in_=std[:])
    m_dt = small.tile([1, TW], dt, tag="m_dt")
    nc.vector.tensor_copy(out=m_dt[:], in_=mean[:])
    return m_dt, rstd


def _bcast_row(nc, pools, row, out_tag="bc"):
    """Broadcast a [1, W] row across 128 partitions -> [128, W] fp32 SBUF,
    via a K=1 PE matmul (ones[1,128].T @ row), in DT for speed."""
    W = row.shape[-1]
    dt = pools["dt"]
    if row.dtype != dt:
        row_dt = pools["small"].tile([1, W], dt, tag=out_tag + "_dtrow")
        nc.vector.tensor_copy(out=row_dt[:], in_=row[:])
        row = row_dt
    ps_full = pools["bc_psum"].tile([P, 512], F32, tag=pools.get("bc_tag", "bc_ps"))
    ps = ps_full[:, :W]
    nc.tensor.matmul(ps[:], pools["ones_row_dt"][:], row[:], start=True, stop=True)
    sb = pools["small"].tile([P, W], F32, tag=out_tag)
    nc.any.tensor_copy(out=sb[:], in_=ps[:])
    return sb


def build_ffn(dt):
    """FFN block: out_partial = relu(ln2(x)_folded @ W1 + b1) @ W2.

    Per-core shard: W1 [E, FS] (FS = 4E/8 = 512 cols), W2 [FS, E].
    Host adds residual + b2 + reduction over cores.
    Inputs: x [B, E, T] DT; w1 [E, FS] DT (g-folded); ws1n [1, FS] DT (-colsum);
            b1c [P, FS//P] f32; w2 [FS, E] DT.  Output: out [B, E, T] f32.
    """
    FS = 4 * E // 8  # 512
    nc = bacc.Bacc("TRN2", target_bir_lowering=False, debug=False)
    x_d = nc.dram_tensor("x", [B, E, T], dt, kind="ExternalInput")
    w1_d = nc.dram_tensor("w1", [E, FS], dt, kind="ExternalInput")
    ws1n_d = nc.dram_tensor("ws1n", [1, FS], dt, kind="ExternalInput")
    b1c_d = nc.dram_tensor("b1c", [P, FS // P], F32, kind="ExternalInput")
    w2_d = nc.dram_tensor("w2", [FS, E], dt, kind="ExternalInput")
    out_d = nc.dram_tensor("out", [B, E, T], F32, kind="ExternalOutput")

    KO = E // P       # 8 k-chunks
    FB = FS // P      # 4 f-blocks
    EB = E // P       # 8 out blocks
    TH = 2            # t halves
    TW = T // TH      # 512

    with tile.TileContext(nc) as tc:
        with (
            tc.tile_pool(name="wts", bufs=1) as wts,
            tc.tile_pool(name="xin", bufs=2) as xin,
            tc.tile_pool(name="mid", bufs=2) as mid,
            tc.tile_pool(name="small", bufs=3) as small,
            tc.tile_pool(name="outp", bufs=3) as outp,
            tc.tile_pool(name="psum", bufs=2, space="PSUM") as psum,
            tc.tile_pool(name="stats_psum", bufs=2, space="PSUM") as stats_psum,
            tc.tile_pool(name="bc_psum", bufs=1, space="PSUM") as bc_psum,
        ):
            pools = {"stats_psum": stats_psum, "small": small, "bc_psum": bc_psum}
            w1_sb = wts.tile([P, KO, FS], dt)
            nc.sync.dma_start(w1_sb[:], w1_d.rearrange("(ko p) f -> p ko f", p=P))
            w2_sb = wts.tile([P, FB, E], dt)
            nc.sync.dma_start(w2_sb[:], w2_d.rearrange("(fo p) e -> p fo e", p=P))
            ws1n_sb = wts.tile([1, FS], dt)
            nc.sync.dma_start(ws1n_sb[:], ws1n_d[:])
            b1_sb = wts.tile([P, FS // P], F32)
            nc.sync.dma_start(b1_sb[:], b1c_d[:])
            ones_dt = wts.tile([P, 1], dt)
            nc.vector.memset(ones_dt[:], 1.0)
            ones_row_dt = wts.tile([1, P], dt)
            nc.vector.memset(ones_row_dt[:], 1.0)
            pools["ones_row_dt"] = ones_row_dt
            pools["dt"] = dt
            eps_tile = wts.tile([1, 1], F32)
            nc.vector.memset(eps_tile[:], EPS)

            for b in range(B):
                for j in range(TH):
                    x_tile = xin.tile([P, KO, TW], dt, tag="x")
                    nc.sync.dma_start(
                        x_tile[:],
                        x_d[b].rearrange("(ko p) t -> p ko t", p=P)[:, :, ts(j, TW)],
                    )
                    xsq = xin.tile([P, KO, TW], dt, tag="xsq")
                    nc.scalar.activation(xsq[:], x_tile[:], AF.Square)
                    m_dt, rstd = _stats_rows(nc, pools, x_tile, xsq, ones_dt, eps_tile, dt)
                    rstd_bc = _bcast_row(nc, pools, rstd, out_tag="rstd_bc")

                    a_sb = mid.tile([P, FB, TW], dt, tag="a")
                    for fb in range(FB):
                        ps = psum.tile([P, TW], F32, tag="ps_a")
                        for ko in range(KO):
                            nc.tensor.matmul(ps[:], w1_sb[:, ko, ts(fb, P)], x_tile[:, ko],
                                             start=(ko == 0), stop=False)
                        nc.tensor.matmul(ps[:], ws1n_sb[:, ts(fb, P)], m_dt[:],
                                         start=False, stop=True)
                        tmp = mid.tile([P, TW], dt, tag="tmp")
                        nc.vector.tensor_mul(tmp[:], ps[:], rstd_bc[:])
                        nc.scalar.activation(a_sb[:, fb], tmp[:], AF.Relu,
                                             bias=b1_sb[:, fb:fb + 1])

                    for eb in range(EB):
                        ps = psum.tile([P, TW], F32, tag="ps_o")
                        for fo in range(FB):
                            nc.tensor.matmul(ps[:], w2_sb[:, fo, ts(eb, P)], a_sb[:, fo],
                                             start=(fo == 0), stop=(fo == FB - 1))
                        o_sb = outp.tile([P, TW], F32, tag="o")
                        nc.any.tensor_copy(out=o_sb[:], in_=ps[:])
                        nc.sync.dma_start(
                            out_d[b, ts(eb, P), ts(j, TW)], o_sb[:])
    nc.compile()
    return nc


# ---------------- host-side helpers ----------------

def ffn_shard_inputs(core, x_f, W1, b1, W2, ln2_g, dt):
    """Prepare per-core input map for the FFN launch.  x_f: [B,E,T] fp32."""
    FS = 4 * E // 8
    ndt = np_dt(dt)
    sl = slice(core * FS, (core + 1) * FS)
    w1 = (ln2_g[:, None] * W1[:, sl]).astype(np.float32)
    ws1n = (-w1.sum(axis=0, keepdims=True))
    w2 = W2[sl, :]
    b1c = b1[sl].reshape(FS // P, P).T.copy()  # [(fb p)] -> [p, fb]
    return {
        "x": x_f.astype(ndt),
        "w1": w1.astype(ndt),
        "ws1n": ws1n.astype(ndt),
        "b1c": b1c.astype(np.float32),
        "w2": w2.astype(ndt),
    }


def ffn_reference(x_f, W1, b1, W2, b2, ln2_g, ln2_b):
    """Full (unsharded) reference of what sum-of-core-outputs (+host residual) should be."""
    x = x_f.transpose(0, 2, 1)  # [B,T,E]
    m = x.mean(-1, keepdims=True)
    v = ((x - m) ** 2).mean(-1, keepdims=True)
    h = (x - m) / np.sqrt(v + EPS) * ln2_g + ln2_b
    out = np.maximum(h @ W1 + b1, 0) @ W2
    return out.transpose(0, 2, 1)  # [B,E,T] (without residual/b2)


SCALE = HD ** -0.5
NMASK = -1.0e9


def build_attn(dt):
    """Attention block for 2 local heads (TP8 over H=16).

    out_partial[b] = att_cat(b) @ Wo_slice   (host adds residual+bo+reduction)

    q/k packed across partitions: q_sb [128, T]: partitions 0:64 = head0 q.T,
    64:128 = head1 q.T (likewise k_sb).  v token-major: v1_sb [128s, 8, 128].
    Causal tiling: for t-half j, s-block i is kept iff i <= 4j+3; diagonal
    (masked) iff i >= 4j.
    """
    nc = bacc.Bacc("TRN2", target_bir_lowering=False, debug=False)
    x_d = nc.dram_tensor("x", [B, E, T], dt, kind="ExternalInput")
    wqk_d = nc.dram_tensor("wqk", [E, 256], dt, kind="ExternalInput")   # [q01 | k01]
    wsqk_d = nc.dram_tensor("wsqk", [1, 256], dt, kind="ExternalInput")  # -colsums
    wv_d = nc.dram_tensor("wv", [E, 128], dt, kind="ExternalInput")      # [v0 | v1]
    wsv_d = nc.dram_tensor("wsv", [1, 128], dt, kind="ExternalInput")
    wo_d = nc.dram_tensor("wo", [128, E], dt, kind="ExternalInput")
    mask_d = nc.dram_tensor("mask", [4, P, T // 2], F32, kind="ExternalInput")
    out_d = nc.dram_tensor("out", [B, E, T], F32, kind="ExternalOutput")

    KO = E // P   # 8
    TH = 2
    TW = T // TH  # 512
    NSB = T // P  # 8 s-blocks

    with tile.TileContext(nc) as tc:
        with (
            tc.tile_pool(name="wts", bufs=1) as wts,
            tc.tile_pool(name="xin", bufs=2) as xin,
            tc.tile_pool(name="perb", bufs=2) as perb,
            tc.tile_pool(name="small", bufs=3) as small,
            tc.tile_pool(name="ew", bufs=5) as ew,
            tc.tile_pool(name="outp", bufs=3) as outp,
            tc.tile_pool(name="pbig", bufs=5, space="PSUM") as pbig,
            tc.tile_pool(name="pacc", bufs=1, space="PSUM") as pacc,
            tc.tile_pool(name="pstat", bufs=2, space="PSUM") as pstat,
        ):
            pools = {"stats_psum": pstat, "small": small, "bc_psum": pbig, "bc_tag": "pb"}
            wqk_sb = wts.tile([P, KO, 256], dt)
            nc.sync.dma_start(wqk_sb[:], wqk_d.rearrange("(ko p) n -> p ko n", p=P))
            wv_sb = wts.tile([P, KO, 128], dt)
            nc.sync.dma_start(wv_sb[:], wv_d.rearrange("(ko p) n -> p ko n", p=P))
            wsqk_sb = wts.tile([1, 256], dt)
            nc.sync.dma_start(wsqk_sb[:], wsqk_d[:])
            wsv_sb = wts.tile([1, 128], dt)
            nc.sync.dma_start(wsv_sb[:], wsv_d[:])
            wo_sb = wts.tile([P, E], dt)
            nc.sync.dma_start(wo_sb[:], wo_d[:])
            mask_sb = wts.tile([P, 4, TW], F32)
            nc.sync.dma_start(mask_sb[:], mask_d.rearrange("d p t -> p d t"))
            ones_dt = wts.tile([P, 1], dt)
            nc.vector.memset(ones_dt[:], 1.0)
            ones_row_dt = wts.tile([1, P], dt)
            nc.vector.memset(ones_row_dt[:], 1.0)
            pools["ones_row_dt"] = ones_row_dt
            pools["dt"] = dt
            eps_tile = wts.tile([1, 1], F32)
            nc.vector.memset(eps_tile[:], EPS)
            ident = wts.tile([P, P], F32)
            from concourse.masks import make_identity
            make_identity(nc, ident[:])

            for b in range(B):
                q_sb = perb.tile([P, T], dt, tag="q")
                k_sb = perb.tile([P, T], dt, tag="k")
                v1_sb = perb.tile([P, NSB, P], dt, tag="v1")
                att_sb = perb.tile([P, T], dt, tag="att")
                for j in range(TH):
                    # ---- load x half, stats ----
                    x_tile = xin.tile([P, KO, TW], dt, tag="x")
                    nc.sync.dma_start(
                        x_tile[:],
                        x_d[b].rearrange("(ko p) t -> p ko t", p=P)[:, :, ts(j, TW)])
                    xsq = xin.tile([P, KO, TW], dt, tag="xsq")
                    nc.scalar.activation(xsq[:], x_tile[:], AF.Square)
                    m_dt, rstd = _stats_rows(nc, pools, x_tile, xsq, ones_dt, eps_tile, dt)
                    rstd_bc = _bcast_row(nc, pools, rstd, out_tag="rstd_bc")

                    # ---- q/k projections (q01 block, k01 block) ----
                    for blk, dst in ((0, q_sb), (1, k_sb)):
                        ps = pbig.tile([P, TW], F32, tag="pb")
                        for ko in range(KO):
                            nc.tensor.matmul(ps[:], wqk_sb[:, ko, ts(blk, P)],
                                             x_tile[:, ko], start=(ko == 0), stop=False)
                        nc.tensor.matmul(ps[:], wsqk_sb[:, ts(blk, P)], m_dt[:],
                                         start=False, stop=True)
                        nc.vector.tensor_mul(dst[:, ts(j, TW)], ps[:], rstd_bc[:])

                    # ---- rstd columns for the 4 t-blocks of this half ----
                    rcol = small.tile([P, 4], F32, tag="rcol")
                    for tb in range(4):
                        pst_full = pbig.tile([P, 512], F32, tag="pb")
                        pst = pst_full[:, :P]
                        nc.tensor.transpose(pst[:], rstd_bc[:, ts(tb, P)], ident[:])
                        nc.any.tensor_copy(out=rcol[:, tb:tb + 1], in_=pst[:, 0:1])

                    # ---- v projection (token-major) ----
                    for tb in range(4):
                        S = 4 * j + tb
                        ps_full = pbig.tile([P, 512], F32, tag="pb")
                        ps = ps_full[:, :P]
                        for ko in range(KO):
                            nc.tensor.matmul(ps[:], x_tile[:, ko, ts(tb, P)],
                                             wv_sb[:, ko], start=(ko == 0), stop=False)
                        nc.tensor.matmul(ps[:], m_dt[:, ts(tb, P)], wsv_sb[:],
                                         start=False, stop=True)
                        nc.vector.tensor_scalar_mul(v1_sb[:, S], ps[:], rcol[:, tb:tb + 1])

                    # ---- attention for this t-half (needs s-blocks 0..4j+3) ----
                    att_ps = pacc.tile([P, TW], F32, tag="ps_att")
                    den0 = pstat.tile([1, TW], F32, tag="ps_small", name="den0")
                    den1 = pstat.tile([1, TW], F32, tag="ps_small", name="den1")
                    dens = (den0, den1)
                    nkept = 4 * j + 4
                    for i in range(nkept):
                        expws = []
                        for h in range(2):
                            hp = slice(h * 64, h * 64 + 64)
                            wei_ps = pbig.tile([P, TW], F32, tag="pb", name=f"wei{h}")
                            nc.tensor.matmul(wei_ps[:], q_sb[hp, ts(i, P)],
                                             k_sb[hp, ts(j, TW)],
                                             start=True, stop=True,
                                             tile_position=(64 * h, 0))
                            expw = ew.tile([P, TW], dt, tag=f"expw{h}")
                            d = i - 4 * j
                            if d >= 0:  # diagonal block: add causal mask first
                                tmp = ew.tile([P, TW], dt, tag=f"masked{h}")
                                nc.vector.tensor_add(tmp[:], wei_ps[:], mask_sb[:, d])
                                nc.scalar.activation(expw[:], tmp[:], AF.Exp, scale=SCALE)
                            else:
                                nc.scalar.activation(expw[:], wei_ps[:], AF.Exp, scale=SCALE)
                            expws.append(expw)
                        for h in range(2):
                            hp = slice(h * 64, h * 64 + 64)
                            nc.tensor.matmul(att_ps[hp, :],
                                             v1_sb[:, i, ts(h, 64)], expws[h][:],
                                             start=(i == 0), stop=(i == nkept - 1),
                                             tile_position=(0, 64 * h),
                                             skip_group_check=True)
                        for h in range(2):
                            nc.tensor.matmul(dens[h][:], ones_dt[:], expws[h][:],
                                             start=(i == 0), stop=(i == nkept - 1))
                    for h in range(2):
                        hp = slice(h * 64, h * 64 + 64)
                        den_sb = small.tile([1, TW], dt, tag="den_sb")
                        nc.any.tensor_copy(out=den_sb[:], in_=dens[h][:])
                        den_bc_ps = pbig.tile([P, TW], F32, tag="pb", name="denbc")
                        nc.tensor.matmul(den_bc_ps[:], ones_row_dt[:], den_sb[:],
                                         start=True, stop=True)
                        recip_bc = small.tile([P, TW], F32, tag="recip_bc")
                        nc.vector.reciprocal_approx_fast(out=recip_bc[:], in_=den_bc_ps[:])
                        nc.vector.tensor_mul(att_sb[hp, ts(j, TW)],
                                             att_ps[hp, :],
                                             recip_bc[hp, :])

                    # ---- Wo projection for this t-half ----
                    for eb in range(KO):
                        ps = pbig.tile([P, TW], F32, tag="pb")
                        nc.tensor.matmul(ps[:], wo_sb[:, ts(eb, P)],
                                         att_sb[:, ts(j, TW)], start=True, stop=True)
                        o_sb = outp.tile([P, TW], F32, tag="o")
                        nc.any.tensor_copy(out=o_sb[:], in_=ps[:])
                        nc.sync.dma_start(out_d[b, ts(eb, P), ts(j, TW)], o_sb[:])
    nc.compile()
    return nc


def attn_shard_inputs(core, x_f, Wq, Wk, Wv, Wo, ln1_g, dt):
    """Per-core inputs for the attention launch.
    Wq/Wk/Wv: [E, H, HD]; Wo: [E, E]; core owns heads 2c, 2c+1."""
    ndt = np_dt(dt)
    h0 = 2 * core
    g = ln1_g[:, None]
    wq = (g * Wq[:, h0:h0 + 2].reshape(E, 128)).astype(np.float32)   # [E, (h,d)]
    wk = (g * Wk[:, h0:h0 + 2].reshape(E, 128)).astype(np.float32)
    wv = (g * Wv[:, h0:h0 + 2].reshape(E, 128)).astype(np.float32)
    wqk = np.concatenate([wq, wk], axis=1)                            # [E, 256]
    wsqk = -wqk.sum(axis=0, keepdims=True)
    wsv = -wv.sum(axis=0, keepdims=True)
    wo = Wo[h0 * HD:(h0 + 2) * HD, :]                                 # [128, E]
    # diagonal causal masks: d = i - 4j in 0..3; allow p <= c - 128d
    masks = np.zeros((4, P, T // 2), np.float32)
    for d in range(4):
        pcol = np.arange(P)[:, None]
        ccol = np.arange(T // 2)[None, :]
        masks[d] = np.where(pcol <= ccol - 128 * d, 0.0, NMASK)
    return {
        "x": x_f.astype(ndt),
        "wqk": wqk.astype(ndt),
        "wsqk": wsqk.astype(ndt),
        "wv": wv.astype(ndt),
        "wsv": wsv.astype(ndt),
        "wo": wo.astype(ndt),
        "mask": masks,
    }


def attn_reference(x_f, Wq, Wk, Wv, Wo, ln1_g, ln1_b):
    """Unsharded reference for sum-of-core attn outputs (no residual/bo)."""
    x = x_f.transpose(0, 2, 1)  # [B,T,E]
    m = x.mean(-1, keepdims=True)
    v_ = ((x - m) ** 2).mean(-1, keepdims=True)
    h = (x - m) / np.sqrt(v_ + EPS) * ln1_g + ln1_b
    q = np.einsum('bte,ehd->bhtd', h, Wq.reshape(E, H, HD))
    k = np.einsum('bte,ehd->bhtd', h, Wk.reshape(E, H, HD))
    v = np.einsum('bte,ehd->bhtd', h, Wv.reshape(E, H, HD))
    wei = np.einsum('bhtd,bhsd->bhts', k, q) * SCALE
    mask = np.tril(np.ones((T, T), bool))
    wei = np.where(mask, wei, -np.inf)
    wei = wei - wei.max(-1, keepdims=True)
    p = np.exp(wei)
    p /= p.sum(-1, keepdims=True)
    att = np.einsum('bhts,bhsd->bhtd', p, v).transpose(0, 2, 1, 3).reshape(B, T, E)
    out = att @ Wo
    return out.transpose(0, 2, 1)  # [B, E, T]


VS = V // 8  # 4000 vocab cols per core
VC = 8       # 500-wide chunks
VW = VS // VC  # 500


def build_lmhead(dt):
    """Final LN + LM head, vocab-sharded: logits[b, t, vshard] (token-major out).

    logits = (x @ Wlm_eff - m[t]*colsum) * rstd[t] + blm
    with lnf_g folded into Wlm_eff and (lnf_b @ Wlm + blm) folded into the bias row.
    """
    nc = bacc.Bacc("TRN2", target_bir_lowering=False, debug=False)
    x_d = nc.dram_tensor("x", [B, E, T], dt, kind="ExternalInput")
    wlm_d = nc.dram_tensor("wlm", [E, VS], dt, kind="ExternalInput")
    wsb_d = nc.dram_tensor("wsb", [2, VS], dt, kind="ExternalInput")  # [-colsums; bias]
    out_d = nc.dram_tensor("out", [B, T, VS], F32, kind="ExternalOutput")

    KO = E // P
    TH = 2
    TW = T // TH

    with tile.TileContext(nc) as tc:
        with (
            tc.tile_pool(name="wts", bufs=1) as wts,
            tc.tile_pool(name="xin", bufs=2) as xin,
            tc.tile_pool(name="small", bufs=3) as small,
            tc.tile_pool(name="outp", bufs=4) as outp,
            tc.tile_pool(name="pall", bufs=6, space="PSUM") as pall,
            tc.tile_pool(name="pstat", bufs=1, space="PSUM") as pstat,
        ):
            pools = {"stats_psum": pstat, "small": small, "bc_psum": pall, "bc_tag": "pb"}
            wlm_sb = wts.tile([P, KO, VS], dt)   # 4 MB bf16
            nc.sync.dma_start(wlm_sb[:], wlm_d.rearrange("(ko p) v -> p ko v", p=P))
            wsb_sb = wts.tile([2, VS], dt)
            nc.sync.dma_start(wsb_sb[:], wsb_d[:])
            ones_dt = wts.tile([P, 1], dt)
            nc.vector.memset(ones_dt[:], 1.0)
            ones_row_dt = wts.tile([1, P], dt)
            nc.vector.memset(ones_row_dt[:], 1.0)
            pools["ones_row_dt"] = ones_row_dt
            pools["dt"] = dt
            eps_tile = wts.tile([1, 1], F32)
            nc.vector.memset(eps_tile[:], EPS)
            ident = wts.tile([P, P], F32)
            from concourse.masks import make_identity
            make_identity(nc, ident[:])

            for b in range(B):
                for j in range(TH):
                    x_tile = xin.tile([P, KO, TW], dt, tag="x")
                    nc.sync.dma_start(
                        x_tile[:],
                        x_d[b].rearrange("(ko p) t -> p ko t", p=P)[:, :, ts(j, TW)])
                    xsq = xin.tile([P, KO, TW], dt, tag="xsq")
                    nc.scalar.activation(xsq[:], x_tile[:], AF.Square)
                    m_dt, rstd = _stats_rows(nc, pools, x_tile, xsq, ones_dt, eps_tile, dt)
                    rstd_bc = _bcast_row(nc, pools, rstd, out_tag="rstd_bc")
                    mo_dt = small.tile([2, T // TH], dt, tag="mo")
                    nc.vector.memset(mo_dt[:], 1.0)
                    nc.vector.tensor_copy(out=mo_dt[0:1, :], in_=m_dt[:])
                    rcol = small.tile([P, 4], F32, tag="rcol")
                    for tb in range(4):
                        pst_full = pall.tile([P, 512], F32, tag="pb")
                        pst = pst_full[:, :P]
                        nc.tensor.transpose(pst[:], rstd_bc[:, ts(tb, P)], ident[:])
                        nc.any.tensor_copy(out=rcol[:, tb:tb + 1], in_=pst[:, 0:1])

                    for tb in range(4):
                        for w in range(2):  # two waves of 4 vocab chunks
                            vcs = range(4 * w, 4 * w + 4)
                            pss = {}
                            for vc in vcs:
                                ps = pall.tile([P, VW], F32, tag="pb", name=f"ps{vc}")
                                pss[vc] = ps
                            for ko in range(KO):
                                for vc in vcs:
                                    nc.tensor.matmul(pss[vc][:], x_tile[:, ko, ts(tb, P)],
                                                     wlm_sb[:, ko, ts(vc, VW)],
                                                     start=(ko == 0), stop=False)
                            for vc in vcs:
                                # K=2 rank-2 update: -m[t]*colsum(W) + 1*bias
                                nc.tensor.matmul(pss[vc][:], mo_dt[:, ts(tb, P)],
                                                 wsb_sb[:, ts(vc, VW)], start=False, stop=True)
                            for vc in vcs:
                                o_sb = outp.tile([P, VW], F32, tag="o")
                                nc.vector.tensor_scalar_mul(o_sb[:], pss[vc][:], rcol[:, tb:tb + 1])
                                nc.sync.dma_start(
                                    out_d[b, ds(j * TW + tb * P, P), ts(vc, VW)], o_sb[:])
    nc.compile()
    return nc


def lmhead_shard_inputs(core, x_f, Wlm, blm, lnf_g, lnf_b, dt):
    ndt = np_dt(dt)
    sl = slice(core * VS, (core + 1) * VS)
    w = (lnf_g[:, None] * Wlm[:, sl]).astype(np.float32)
    ws = -w.sum(axis=0, keepdims=True)
    bias = (blm[sl] + lnf_b @ Wlm[:, sl]).astype(np.float32)[None, :]
    return {
        "x": x_f.astype(ndt),
        "wlm": w.astype(ndt),
        "wsb": np.concatenate([ws, bias], axis=0).astype(ndt),
    }


# ====================== host orchestration ======================

_programs = None
last_exec_ns = 0


def _get_programs(dt):
    global _programs
    if _programs is None:
        _programs = (build_attn(dt), build_ffn(dt), build_lmhead(dt))
    return _programs


def kernel(**inputs):
    """Full-model forward.  Takes setup_inputs() arrays, returns logits [B,T,V] f32."""
    global last_exec_ns
    from concourse.bass_utils import run_bass_kernel_spmd
    dt = mybir.dt.bfloat16
    trace = bool(int(os.environ.get("TRN_LLM_TRACE", "0")))

    idx = np.asarray(inputs["idx"])
    f32 = lambda k: np.asarray(inputs[k], dtype=np.float32)
    tok_emb, pos_emb = f32("tok_emb"), f32("pos_emb")
    Wq, Wk, Wv, Wo, bo = f32("Wq"), f32("Wk"), f32("Wv"), f32("Wo"), f32("bo")
    W1, b1, W2, b2 = f32("W1"), f32("b1"), f32("W2"), f32("b2")
    ln1_g, ln1_b = f32("ln1_g"), f32("ln1_b")
    ln2_g, ln2_b = f32("ln2_g"), f32("ln2_b")
    lnf_g, lnf_b = f32("lnf_g"), f32("lnf_b")
    Wlm, blm = f32("Wlm"), f32("blm")
    assert np.all(ln1_b == 0), "nonzero ln1 beta requires the slow path (not built)"

    attn_nc, ffn_nc, lm_nc = _get_programs(dt)
    cores = list(range(8))
    exec_ns = 0

    def run(nc, in_maps):
        nonlocal exec_ns
        res = run_bass_kernel_spmd(nc, in_maps, cores, trace=trace)
        if trace and res.exec_time_ns:
            exec_ns += res.exec_time_ns
        return res

    # embedding on host; feature-major residual stream [B, E, T]
    x = (tok_emb[idx] + pos_emb[None, :T]).transpose(0, 2, 1).astype(np.float32)
    x = np.ascontiguousarray(x)

    for l in range(L):
        maps = [attn_shard_inputs(c, x, Wq[l], Wk[l], Wv[l], Wo[l], ln1_g[l], dt)
                for c in cores]
        res = run(attn_nc, maps)
        x = x + np.sum([r["out"] for r in res.results], axis=0) + bo[l][None, :, None]
        maps = [ffn_shard_inputs(c, x, W1[l], b1[l] + ln2_b[l] @ W1[l], W2[l], ln2_g[l], dt)
                for c in cores]
        res = run(ffn_nc, maps)
        x = x + np.sum([r["out"] for r in res.results], axis=0) + b2[l][None, :, None]

    maps = [lmhead_shard_inputs(c, x, Wlm, blm, lnf_g, lnf_b, dt) for c in cores]
    res = run(lm_nc, maps)
    logits = np.empty((B, T, V), np.float32)
    for c in cores:
        logits[:, :, c * VS:(c + 1) * VS] = res.results[c]["out"]
    last_exec_ns = exec_ns
    return logits



# revision 11
# speedup vs baseline: 5.6272x; 5.6272x over previous
"""Trainium2 Bass kernel for the 6-layer transformer LM (B=4, T=1024, E=1024,
H=16, V=32000) on 8 NeuronCores.

Strategy: data-parallel over tokens, single kernel launch for the whole model.
Each core owns 512 tokens (batch b = core//2; 4 interleaved 128-token blocks:
{0,3,4,7} for even cores, {1,2,5,6} for odd). All weights are replicated and
streamed from HBM. Attention needs q/v of the sibling core's tokens: after
LayerNorm the normalized hidden state h (bf16, 1MB) is exchanged with a
pair-wise device AllGather (~16us, hidden under the k projection); each core
then computes q/v for all 1024 tokens of its batch element (redundant +2
projections) and full causal attention for its own 512 query positions.
FFN, final LN and the LM head are purely local (per-token). Host does the
embedding gather up front and reassembles logits blocks at the end.

LayerNorm gains are folded into the following matmul's weights host-side;
lnf_b is folded into the LM-head bias. ln1_b/ln2_b must be zero (asserted).
Matmuls in bf16 (fp32 PSUM accumulation); softmax/stats in fp32.
"""
import os
import sys
sys.path.insert(0, "/opt/trn_rl_repo")

import numpy as np
import ml_dtypes

import concourse.bacc as bacc
import concourse.tile as tile
from concourse import mybir
from concourse.bass import ts, ds

P = 128
B, T, E, H, HD, V, L = 4, 1024, 1024, 16, 64, 32000, 6
KO = E // P            # 8 contraction chunks
TW = 512               # tokens per core
NB = T // P            # 8 token blocks per batch element
VB = V // P            # 250 vocab blocks
FF = 4 * E             # 4096
EPS = 1e-5
SCALE = HD ** -0.5
NMASK = -1.0e9

LOCAL_BLOCKS = [[0, 3, 4, 7], [1, 2, 5, 6]]   # per sub-core token-block sets
GO = [0, 3, 4, 7, 1, 2, 5, 6]                 # gathered (pair-concat) block order
POS = [GO.index(j) for j in range(8)]          # s-block j -> gathered position
SUF_W = [512, 512, 384, 384, 256, 256, 128, 128]  # union suffix widths
SUF_S = [0, 0, 128, 128, 256, 256, 384, 384]      # union suffix starts
# per s-block j, exactly one 128-col block (the first of the suffix) may need
# masking (diagonal or over-causal); it sits at packed t-slot SUF_S[j]//128.

DT = mybir.dt.bfloat16
F32 = mybir.dt.float32
AF = mybir.ActivationFunctionType
OP = mybir.AluOpType


def np_dt(dt):
    return ml_dtypes.bfloat16 if dt == mybir.dt.bfloat16 else np.float32


def build_model(n_layers=L):
    nc = bacc.Bacc("TRN2", target_bir_lowering=False, num_devices=8)
    n_wl = max(1, n_layers)

    x0_d = nc.dram_tensor("x0", [P, KO, TW], F32, kind="ExternalInput")
    wq_d = nc.dram_tensor("wq", [n_wl, P, KO, E], DT, kind="ExternalInput")
    wk_d = nc.dram_tensor("wk", [n_wl, P, KO, E], DT, kind="ExternalInput")
    wv_d = nc.dram_tensor("wv", [n_wl, P, KO, E], DT, kind="ExternalInput")
    wo_d = nc.dram_tensor("wo", [n_wl, P, KO, E], DT, kind="ExternalInput")
    w1_d = nc.dram_tensor("w1", [n_wl, P, KO, FF], DT, kind="ExternalInput")
    w2_d = nc.dram_tensor("w2", [n_wl, P, FF // P, E], DT, kind="ExternalInput")
    wlm_d = nc.dram_tensor("wlm", [P, KO, V], DT, kind="ExternalInput")
    mask_d = nc.dram_tensor("mask", [8, P, P], DT, kind="ExternalInput")
    bo_d = nc.dram_tensor("bo", [n_wl, P, KO], F32, kind="ExternalInput")
    b1_d = nc.dram_tensor("b1", [n_wl, P, FF // P], F32, kind="ExternalInput")
    b2_d = nc.dram_tensor("b2", [n_wl, P, KO], F32, kind="ExternalInput")
    blm_d = nc.dram_tensor("blm", [P, VB], F32, kind="ExternalInput")
    out_d = nc.dram_tensor("out", [VB, P, TW], DT, kind="ExternalOutput")
    xout_d = nc.dram_tensor("xout", [P, KO, TW], F32, kind="ExternalOutput")

    with tile.TileContext(nc) as tc:
        with (
            tc.tile_pool(name="cst", bufs=1) as cst,
            tc.tile_pool(name="xp", bufs=1) as xp,
            tc.tile_pool(name="lnp", bufs=1) as lnp,       # xb/att, xsq/k, h
            tc.tile_pool(name="bigp", bufs=1) as bigp,     # h_all / ffn act
            tc.tile_pool(name="qp", bufs=1) as qp,
            tc.tile_pool(name="vp", bufs=1) as vp,
            tc.tile_pool(name="ewp", bufs=3) as ewp,       # expw etc
            tc.tile_pool(name="smallp", bufs=1) as smallp,
            tc.tile_pool(name="biasp", bufs=2) as biasp,
            tc.tile_pool(name="outp", bufs=2) as outp,
            tc.tile_pool(name="dramp", bufs=2, space="DRAM") as dramp,
            tc.tile_pool(name="pp", bufs=3, space="PSUM") as pp,
            tc.tile_pool(name="pa", bufs=2, space="PSUM") as pa,
            tc.tile_pool(name="pbc", bufs=2, space="PSUM") as pbc,
        ):
            # ---- constants ----
            ones_col = cst.tile([P, 1], DT)
            nc.vector.memset(ones_col[:], 1.0)
            ones_row = cst.tile([1, P], DT)
            nc.vector.memset(ones_row[:], 1.0)
            eps_t = cst.tile([1, 1], F32)
            nc.vector.memset(eps_t[:], EPS)
            mask_sb = cst.tile([P, 8, P], DT)
            nc.sync.dma_start(mask_sb[:], mask_d.rearrange("j p c -> p j c"))
            blm_sb = cst.tile([P, VB], F32)
            nc.sync.dma_start(blm_sb[:], blm_d[:])

            # ---- residual stream (fp32, resident) ----
            x_sb = xp.tile([P, KO, TW], F32)
            nc.sync.dma_start(x_sb[:], x0_d[:])

            def ln_normalize(h_dst):
                """LN stats over the local x strip; writes normalized bf16 h
                (gain folded into the following weights host-side)."""
                xb = lnp.tile([P, KO, TW], DT, tag="xb", name="xb")
                nc.vector.tensor_copy(out=xb[:], in_=x_sb[:])
                xsq = lnp.tile([P, KO, TW], DT, tag="xsq", name="xsq")
                nc.scalar.activation(xsq[:], xb[:], AF.Square)
                ps_sum = pbc.tile([1, TW], F32, tag="pbc", name="ps_sum")
                ps_sq = pbc.tile([1, TW], F32, tag="pbc", name="ps_sq")
                for ko in range(KO):
                    nc.tensor.matmul(ps_sum[:], ones_col[:], xb[:, ko],
                                     start=(ko == 0), stop=(ko == KO - 1))
                for ko in range(KO):
                    nc.tensor.matmul(ps_sq[:], ones_col[:], xsq[:, ko],
                                     start=(ko == 0), stop=(ko == KO - 1))
                inv = 1.0 / E
                mean = smallp.tile([1, TW], F32, tag="stat", name="mean", bufs=3)
                nc.vector.tensor_scalar_mul(mean[:], ps_sum[:], inv)
                m_dt = smallp.tile([1, TW], DT, tag="m_dt", name="m_dt")
                nc.vector.tensor_copy(out=m_dt[:], in_=mean[:])
                var = smallp.tile([1, TW], F32, tag="stat", name="var", bufs=3)
                nc.vector.tensor_scalar_mul(var[:], ps_sq[:], inv)
                msq = smallp.tile([1, TW], F32, tag="stat", name="msq", bufs=3)
                nc.vector.tensor_mul(msq[:], mean[:], mean[:])
                nc.vector.tensor_sub(var[:], var[:], msq[:])
                std = smallp.tile([1, TW], F32, tag="stat", name="std", bufs=3)
                nc.scalar.activation(std[:], var[:], AF.Sqrt, bias=eps_t[:1])
                rstd = smallp.tile([1, TW], F32, tag="stat", name="rstd", bufs=3)
                nc.vector.reciprocal_approx_fast(out=rstd[:], in_=std[:])
                r_dt = smallp.tile([1, TW], DT, tag="r_dt", name="r_dt")
                nc.vector.tensor_copy(out=r_dt[:], in_=rstd[:])
                mb = pbc.tile([P, TW], F32, tag="pbc", name="mb")
                nc.tensor.matmul(mb[:], ones_row[:], m_dt[:], start=True, stop=True)
                rb = pbc.tile([P, TW], F32, tag="pbc", name="rb")
                nc.tensor.matmul(rb[:], ones_row[:], r_dt[:], start=True, stop=True)
                for ko in range(KO):
                    ntmp = ewp.tile([P, TW], DT, tag="ntmp", name="ntmp")
                    nc.vector.tensor_sub(ntmp[:], xb[:, ko], mb[:])
                    nc.vector.tensor_mul(h_dst[:, ko], ntmp[:], rb[:])

            def layer_body(l, wap, wfp, w2p):
                if True:
                    # ---- LN1 + pair AllGather of h ----
                    h_loc = lnp.tile([P, KO, TW], DT, tag="hln", name="h_loc")
                    ln_normalize(h_loc)
                    bounce = dramp.tile([P, KO, TW], DT, tag="bounce",
                                        name="bounce")
                    nc.sync.dma_start(bounce[:], h_loc[:])
                    gath = dramp.tile([2, P, KO, TW], DT, tag="gath", name="gath")
                    nc.gpsimd.collective_compute(
                        "AllGather", OP.bypass,
                        replica_groups=[[0, 1], [2, 3], [4, 5], [6, 7]],
                        ins=[bounce[:]], outs=[gath[:]],
                    )

                    # ---- k projection from local h (overlaps the AllGather) ----
                    wk_sb = wap.tile([P, KO, E], DT, tag="wa", name="wk_sb")
                    nc.sync.dma_start(wk_sb[:], wk_d[l])
                    k_sb = lnp.tile([P, KO, TW], DT, tag="xsq", name="k_sb")
                    for hp in range(8):
                        pk = pp.tile([P, TW], F32, tag="pp", name="pk")
                        for ko in range(KO):
                            nc.tensor.matmul(pk[:], wk_sb[:, ko, ts(hp, P)],
                                             h_loc[:, ko], start=(ko == 0),
                                             stop=(ko == KO - 1))
                        nc.vector.tensor_copy(out=k_sb[:, hp], in_=pk[:])

                    h_all = bigp.tile([P, KO, 2, TW], DT, tag="big", name="h_all")
                    for r in range(2):
                        nc.sync.dma_start(h_all[:, :, r], gath[r])

                    # ---- q projection (all 1024 tokens of the pair) ----
                    wq_sb = wap.tile([P, KO, E], DT, tag="wa", name="wq_sb")
                    nc.sync.dma_start(wq_sb[:], wq_d[l])
                    q_sb = qp.tile([P, 8, 2, TW], DT, name="q_sb")
                    for hp in range(8):
                        for r in range(2):
                            pq = pp.tile([P, TW], F32, tag="pp", name="pq")
                            for ko in range(KO):
                                nc.tensor.matmul(pq[:], wq_sb[:, ko, ts(hp, P)],
                                                 h_all[:, ko, r], start=(ko == 0),
                                                 stop=(ko == KO - 1))
                            nc.vector.tensor_copy(out=q_sb[:, hp, r], in_=pq[:])

                    # ---- v projection (token-major, with fused ones column) ----
                    wv_sb = wap.tile([P, KO, E], DT, tag="wa", name="wv_sb")
                    nc.sync.dma_start(wv_sb[:], wv_d[l])
                    v_sb = vp.tile([P, 8, H, HD + 1], DT, name="v_sb")
                    for g in range(8):
                        r, tb = g // 4, g % 4
                        for vh in range(2):
                            pv = pp.tile([P, TW], F32, tag="pp", name="pv")
                            for ko in range(KO):
                                nc.tensor.matmul(
                                    pv[:], h_all[:, ko, r, ts(tb, P)],
                                    wv_sb[:, ko, ts(vh, TW)],
                                    start=(ko == 0), stop=(ko == KO - 1))
                            nc.vector.tensor_copy(
                                out=v_sb[:, g, vh * 8:(vh + 1) * 8, 0:HD],
                                in_=pv[:].rearrange("p (h d) -> p h d", h=8))
                    nc.vector.memset(v_sb[:, :, :, HD:HD + 1], 1.0)

                    # ---- attention: per head, union causal suffixes ----
                    att_sb = lnp.tile([P, KO, TW], DT, tag="xb", name="att_sb")
                    for h in range(H):
                        hrow = 64 * (h % 2)
                        hp = h // 2
                        att_ps = pa.tile([P, TW], F32, tag="pa", name="att_ps")
                        for j in range(8):
                            W, S = SUF_W[j], SUF_S[j]
                            psc = pp.tile([P, TW], F32, tag="pp", name="psc")
                            nc.tensor.matmul(
                                psc[:, 0:W],
                                q_sb[hrow:hrow + 64, hp, POS[j] // 4,
                                     ts(POS[j] % 4, P)],
                                k_sb[hrow:hrow + 64, hp, S:S + W],
                                start=True, stop=True,
                                tile_position=(hrow, 0))
                            expw = ewp.tile([P, TW], DT, tag="expw", name="expw")
                            mtmp = ewp.tile([P, P], DT, tag="mtmp", name="mtmp")
                            nc.vector.tensor_add(mtmp[:], psc[:, 0:P],
                                                 mask_sb[:, j])
                            nc.scalar.activation(expw[:, 0:P], mtmp[:], AF.Exp,
                                                 scale=SCALE)
                            if W > P:
                                nc.scalar.activation(expw[:, P:W], psc[:, P:W],
                                                     AF.Exp, scale=SCALE)
                            nc.tensor.matmul(att_ps[0:HD + 1, S:S + W],
                                             v_sb[:, POS[j], h],
                                             expw[:, 0:W],
                                             start=(j == 0), stop=(j == 7),
                                             skip_group_check=True)
                        recip = smallp.tile([1, TW], F32, tag="recip",
                                            name="recip", bufs=2)
                        nc.vector.reciprocal_approx_fast(
                            out=recip[:], in_=att_ps[HD:HD + 1, :])
                        recip_dt = smallp.tile([1, TW], DT, tag="recip_dt",
                                               name="recip_dt", bufs=2)
                        nc.vector.tensor_copy(out=recip_dt[:], in_=recip[:])
                        rbc = pbc.tile([P, TW], F32, tag="pbc", name="rbc")
                        nc.tensor.matmul(rbc[0:HD, :], ones_row[:, 0:HD],
                                         recip_dt[:], start=True, stop=True)
                        araw = ewp.tile([HD, TW], DT, tag="araw", name="araw")
                        nc.scalar.activation(araw[:], att_ps[0:HD, :], AF.Copy)
                        nc.vector.tensor_mul(att_sb[hrow:hrow + 64, hp],
                                             araw[:], rbc[0:HD, :])

                    # ---- output projection + residual (+bo) ----
                    wo_sb = wap.tile([P, KO, E], DT, tag="wa", name="wo_sb")
                    nc.sync.dma_start(wo_sb[:], wo_d[l])
                    bo_sb = biasp.tile([P, KO], F32, tag="bo", name="bo_sb")
                    nc.sync.dma_start(bo_sb[:], bo_d[l])
                    for eb in range(KO):
                        po = pp.tile([P, TW], F32, tag="pp", name="po")
                        for hp in range(8):
                            nc.tensor.matmul(po[:], wo_sb[:, hp, ts(eb, P)],
                                             att_sb[:, hp], start=(hp == 0),
                                             stop=(hp == 7))
                        nc.vector.scalar_tensor_tensor(
                            out=x_sb[:, eb], in0=po[:],
                            scalar=bo_sb[:, eb:eb + 1],
                            in1=x_sb[:, eb], op0=OP.add, op1=OP.add)

                    # ---- FFN ----
                    h2 = lnp.tile([P, KO, TW], DT, tag="hln", name="h2")
                    ln_normalize(h2)
                    b1_sb = biasp.tile([P, FF // P], F32, tag="b1", name="b1_sb")
                    nc.sync.dma_start(b1_sb[:], b1_d[l])
                    a_sb = bigp.tile([P, FF // P, TW], DT, tag="big", name="a_sb")
                    for fc in range(8):
                        w1_sb = wfp.tile([P, KO, 512], DT, tag="wf", name="w1_sb")
                        nc.sync.dma_start(w1_sb[:], w1_d[l][:, :, ts(fc, 512)])
                        for fb in range(4):
                            f = fc * 4 + fb
                            pf = pp.tile([P, TW], F32, tag="pp", name="pf")
                            for ko in range(KO):
                                nc.tensor.matmul(pf[:], w1_sb[:, ko, ts(fb, P)],
                                                 h2[:, ko], start=(ko == 0),
                                                 stop=(ko == KO - 1))
                            nc.scalar.activation(a_sb[:, f], pf[:], AF.Relu,
                                                 bias=b1_sb[:, f:f + 1])
                    b2_sb = biasp.tile([P, KO], F32, tag="b2", name="b2_sb")
                    nc.sync.dma_start(b2_sb[:], b2_d[l])
                    for eb in range(KO):
                        po = pp.tile([P, TW], F32, tag="pp", name="po2")
                        for fh in range(2):
                            w2_sb = w2p.tile([P, 16, P], DT, tag="w2",
                                             name="w2_sb")
                            nc.sync.dma_start(
                                w2_sb[:], w2_d[l][:, ds(fh * 16, 16), ts(eb, P)])
                            for fo in range(16):
                                nc.tensor.matmul(po[:], w2_sb[:, fo],
                                                 a_sb[:, fh * 16 + fo],
                                                 start=(fh == 0 and fo == 0),
                                                 stop=(fh == 1 and fo == 15))
                        nc.vector.scalar_tensor_tensor(
                            out=x_sb[:, eb], in0=po[:],
                            scalar=b2_sb[:, eb:eb + 1],
                            in1=x_sb[:, eb], op0=OP.add, op1=OP.add)

            with (
                tc.tile_pool(name="wap", bufs=2) as wap,
                tc.tile_pool(name="wfp", bufs=2) as wfp,
                tc.tile_pool(name="w2p", bufs=2) as w2p,
            ):
                for l in range(n_layers):
                    layer_body(l, wap, wfp, w2p)

            nc.sync.dma_start(xout_d[:], x_sb[:])

            # ---- final LN + LM head (local tokens, full vocab) ----
            with tc.tile_pool(name="wlmp", bufs=3) as wlmp:
                hf = lnp.tile([P, KO, TW], DT, tag="hln", name="hf")
                ln_normalize(hf)
                off = 0
                while off < V:
                    cw = min(512, V - off)
                    wlm_sb = wlmp.tile([P, KO, 512], DT, tag="wlm", name="wlm_sb")
                    nc.sync.dma_start(wlm_sb[:, :, 0:cw], wlm_d[:, :, ds(off, cw)])
                    for vb in range(cw // P):
                        gvb = off // P + vb
                        plm = pp.tile([P, TW], F32, tag="pp", name="plm")
                        for ko in range(KO):
                            nc.tensor.matmul(plm[:], wlm_sb[:, ko, ts(vb, P)],
                                             hf[:, ko], start=(ko == 0),
                                             stop=(ko == KO - 1))
                        ob = outp.tile([P, TW], DT, tag="ob", name="ob")
                        nc.scalar.activation(ob[:], plm[:], AF.Identity,
                                             bias=blm_sb[:, gvb:gvb + 1])
                        nc.sync.dma_start(out_d[gvb], ob[:])
                    off += cw
    nc.compile()
    return nc


# ====================== host orchestration ======================

_program = None
last_exec_ns = 0
N_LAYERS = L          # debug knob: run only the first N layers


def _get_program():
    global _program
    if _program is None:
        _program = build_model(N_LAYERS)
    return _program


def _host_weights(inputs):
    """Fold LN gains, reshape weights to the device layouts (shared by cores)."""
    ndt = np_dt(DT)
    f32 = lambda k: np.asarray(inputs[k], dtype=np.float32)
    Wq, Wk, Wv, Wo = f32("Wq"), f32("Wk"), f32("Wv"), f32("Wo")
    W1, W2 = f32("W1"), f32("W2")
    ln1_g, ln1_b = f32("ln1_g"), f32("ln1_b")
    ln2_g, ln2_b = f32("ln2_g"), f32("ln2_b")
    lnf_g, lnf_b = f32("lnf_g"), f32("lnf_b")
    Wlm, blm = f32("Wlm"), f32("blm")
    assert np.all(ln1_b == 0) and np.all(ln2_b == 0), "nonzero ln betas unsupported"

    def proj(w):  # [L, E, M] -> [L, 128, KO, M]
        Lx, _, M = w.shape
        return np.ascontiguousarray(
            w.reshape(Lx, KO, P, M).transpose(0, 2, 1, 3)).astype(ndt)

    wq = proj(ln1_g[:, :, None] * Wq.reshape(L, E, E))
    wk = proj(ln1_g[:, :, None] * Wk.reshape(L, E, E))
    wv = proj(ln1_g[:, :, None] * Wv.reshape(L, E, E))
    wo = proj(Wo)
    w1 = proj(ln2_g[:, :, None] * W1)
    w2 = np.ascontiguousarray(
        W2.reshape(L, FF // P, P, E).transpose(0, 2, 1, 3)).astype(ndt)
    wlm = np.ascontiguousarray(
        (lnf_g[:, None] * Wlm).reshape(KO, P, V).transpose(1, 0, 2)).astype(ndt)
    blm_eff = (blm + lnf_b @ Wlm).astype(np.float32)
    blm_h = np.ascontiguousarray(blm_eff.reshape(VB, P).T)
    bo_h = np.ascontiguousarray(f32("bo").reshape(L, KO, P).transpose(0, 2, 1))
    b1_h = np.ascontiguousarray(
        f32("b1").reshape(L, FF // P, P).transpose(0, 2, 1)).astype(np.float32)
    b2_h = np.ascontiguousarray(f32("b2").reshape(L, KO, P).transpose(0, 2, 1))
    return dict(wq=wq, wk=wk, wv=wv, wo=wo, w1=w1, w2=w2, wlm=wlm,
                blm=blm_h, bo=bo_h, b1=b1_h, b2=b2_h)


def _host_masks(sub):
    """Per-core mask tiles [8, 128, 128] for the (single) maskable block of
    each s-block's union suffix."""
    lb = LOCAL_BLOCKS[sub]
    m = np.zeros((8, P, P), np.float32)
    for j in range(8):
        slot = SUF_S[j] // P
        g = lb[slot]
        if g > j:
            continue                      # fully allowed
        elif g < j:
            m[j, :, :] = NMASK            # over-causal: fully masked
        else:                             # diagonal: allow s_row <= t_col
            sr = np.arange(P)[:, None]
            tc_ = np.arange(P)[None, :]
            m[j] = np.where(sr <= tc_, 0.0, NMASK)
    return m.astype(np_dt(DT))


def kernel(**inputs):
    global last_exec_ns
    from concourse.bass_utils import run_bass_kernel_spmd
    trace = bool(int(os.environ.get("TRN_LLM_TRACE", "0")))

    idx = np.asarray(inputs["idx"])
    tok_emb = np.asarray(inputs["tok_emb"], dtype=np.float32)
    pos_emb = np.asarray(inputs["pos_emb"], dtype=np.float32)
    shared = _host_weights(inputs)
    masks = [_host_masks(0), _host_masks(1)]

    if N_LAYERS != L:                      # debug: truncate per-layer weights
        for kk in ("wq", "wk", "wv", "wo", "w1", "w2", "bo", "b1", "b2"):
            shared[kk] = np.ascontiguousarray(shared[kk][:max(1, N_LAYERS)])

    emb = tok_emb[idx] + pos_emb[None, :T]      # [B, T, E] fp32
    maps = []
    for c in range(8):
        b, sub = c // 2, c % 2
        lb = LOCAL_BLOCKS[sub]
        xs = np.concatenate([emb[b, g * P:(g + 1) * P] for g in lb], axis=0)
        x0 = np.ascontiguousarray(
            xs.T.reshape(KO, P, TW).transpose(1, 0, 2)).astype(np.float32)
        m = dict(shared)
        m["x0"] = x0
        m["mask"] = masks[sub]
        maps.append(m)

    nc = _get_program()
    res = run_bass_kernel_spmd(nc, maps, list(range(8)), trace=trace)
    last_exec_ns = res.exec_time_ns or 0
    global last_results
    last_results = res.results

    logits = np.empty((B, T, V), np.float32)
    for c in range(8):
        b, sub = c // 2, c % 2
        lb = LOCAL_BLOCKS[sub]
        out = np.asarray(res.results[c]["out"], dtype=np.float32)  # [VB,128,TW]
        out = out.transpose(2, 0, 1).reshape(TW, V)                # [TW, V]
        for p, g in enumerate(lb):
            logits[b, g * P:(g + 1) * P] = out[p * P:(p + 1) * P]
    return logits
